# revision 90
# baseline (speedup 1.0000x reference)
"""Causal self-attention Bass kernel for 8 TRN2 NeuronCores.

Problem: B=4, T=2048, C=1024, H=16 heads, head_dim=64, fp32.
    q = x @ Wq.T ; k = x @ Wk.T ; v = x @ Wv.T          (per head)
    att = softmax(mask(q k^T / 8))
    y = att @ v ; out = y @ Wp.T

Sharding (8 cores): 4-way data parallel over batch x 2-way tensor
parallel over heads. Core c handles batch c//2 and heads 8*(c%2)..+8.
Wq/Wk/Wv column-parallel, Wp row-parallel; the partial outputs of the
two head-halves of each batch are summed on the host (the "all-reduce"
of row-parallel Wp).

Pipeline design (v7):
  - Projections stream xT by 512-wide t-chunks (tch).  The causal
    structure means attention q-chunk qc only needs projections from
    chunks <= qc, so attention blocks INTERLEAVE with projection
    blocks: tch0, tch1, qc0, tch2, qc1, tch3, qc2, qc3.  The
    projection blocks are PE-bound while attention is ACT/DVE-heavy,
    so interleaving keeps every engine fed.
  - exp alternates between ACT (even kt, true exp) and DVE (odd kt,
    Schraudolph bf16 bit-hack: bits = round(s*23.083 + 16248.5) as
    int16 reinterpreted bf16; ~1.8% rms on half the weights, ~5e-3
    added rel err after softmax averaging).
  - PV lags exp by 3 kts; scores/PV emitted in 2-kt batches to halve
    rhs-stream-switch pipeline refills.
  - Diagonal kts run FIRST within each (qc, pr) so the serial
    exp->mask(gpsimd)->PV chain overlaps the pr-start bubble and the
    pr tail drains mask-free.
  - Output projection chains interleave into later attention blocks.
  - Softmax normalize: sums (ones-column of the PV stationary) ->
    ACT/DVE copies release the psum accumulators early -> DVE
    reciprocal -> DRAM-bounce broadcast -> gpsimd scale (deferred one
    pr so gpsimd's FIFO doesn't delay causal masks).
  - Everything bf16: FWL keeps LDWEIGHTS off the critical path, input
    DMA traffic halves.  fp32 accumulation throughout.
"""

from contextlib import ExitStack

import numpy as np

import concourse.bass as bass
import concourse.tile as tile
from concourse import bacc, mybir

F32 = mybir.dt.float32
BF16 = mybir.dt.bfloat16
I16 = mybir.dt.int16

B, T, C, H, D = 4, 2048, 1024, 16, 64
NCORES = 8
JL = 512            # local j dims per core (8 heads * 64)
NPAIR = 4           # local head pairs
CI = C // 128       # 8 c-tiles
NT = T // 128       # 16 t/k tiles
NQC = T // 512      # 4 q chunks
VW = D + 1

# Schraudolph bf16 bit-hack exp: bits = s*0.125*log2(e)*128 + (127*128 - 7.5)
EXP_A = 0.125 * 1.4426950408889634 * 128.0
EXP_B = 127.0 * 128.0 - 7.5

_CACHED_NC = None


def build_nc():
    nc = bacc.Bacc(None)

    xT = nc.dram_tensor("xT", [C, T], BF16, kind="ExternalInput")
    wqT = nc.dram_tensor("wqT", [C, JL], BF16, kind="ExternalInput")
    wkT = nc.dram_tensor("wkT", [C, JL], BF16, kind="ExternalInput")
    wvT = nc.dram_tensor("wvT", [C, JL], BF16, kind="ExternalInput")
    wpT = nc.dram_tensor("wpT", [JL, C], BF16, kind="ExternalInput")
    out = nc.dram_tensor("out", [T, C], BF16, kind="ExternalOutput")
    # bounce buffer for broadcasting softmax reciprocals across partitions
    rcd = nc.dram_tensor("rcd", [NPAIR, NQC, 2, 512], F32)

    xT_r = xT.rearrange("(ci p) t -> p ci t", p=128)
    wq_r = wqT.rearrange("(ci p) j -> p ci j", p=128)
    wk_r = wkT.rearrange("(ci p) j -> p ci j", p=128)
    wv_r = wvT.rearrange("(ci p) j -> p ci j", p=128)
    wp_r = wpT.rearrange("(ji p) c -> p ji c", p=128)

    with tile.TileContext(nc) as tc, ExitStack() as ctx:
        pm = ctx.enter_context(tc.tile_pool(name="pm", bufs=1))
        qkp = ctx.enter_context(tc.tile_pool(name="qkp", bufs=1))
        expp = ctx.enter_context(tc.tile_pool(name="expp", bufs=5))
        bcp = ctx.enter_context(tc.tile_pool(name="bcp", bufs=2))
        rcp = ctx.enter_context(tc.tile_pool(name="rcp", bufs=2))
        sab = ctx.enter_context(tc.tile_pool(name="sab", bufs=2))
        stp = ctx.enter_context(tc.tile_pool(name="stp", bufs=2))
        outp = ctx.enter_context(tc.tile_pool(name="outp", bufs=3))
        wpool = ctx.enter_context(tc.tile_pool(name="ph1w", bufs=1))
        xpool = ctx.enter_context(tc.tile_pool(name="ph1x", bufs=2))
        # PSUM: scores (f32, 2 banks each) x3 + shared f32 ring x2 = 8 banks
        gp = ctx.enter_context(tc.tile_pool(name="gp", bufs=3, space="PSUM"))
        psf = ctx.enter_context(tc.tile_pool(name="psf", bufs=2, space="PSUM"))

        # v with a ones column prepended per head (so the softmax sums land
        # on psum partition 0), plus 64 pad columns so every per-head lhsT
        # can be read as [128, 128] -- NumWeights==128 enables FWL.
        v_sb = pm.tile([128, NT, 8 * VW + 64], BF16)
        v_view = v_sb[:, :, 0 : 8 * VW].rearrange("p n (h w) -> p n h w", w=VW)
        ones_col = pm.tile([128, NT, 8, 1], F32)
        nc.vector.memset(ones_col[:], 1.0)
        nc.vector.tensor_copy(v_view[:, :, :, 0:1], ones_col[:])
        nc.vector.memset(v_sb[:, :, 8 * VW : 8 * VW + 64], 0.0)
        # [1, 65] ones: stationary operand of the contraction-1 broadcast
        # matmul used by the last normalize
        ones_bc = pm.tile([1, D + 1], BF16)
        nc.vector.memset(ones_bc[:], 1.0)

        qT_all = qkp.tile([128, NPAIR, T], BF16, tag="qT_all")
        kT_all = qkp.tile([128, NPAIR, T], BF16, tag="kT_all")
        # one yT tile per head-pair: precise dependency tracking, so an
        # out-proj chain's ji<3 matmuls never wait on a fresher pr's yT
        yTs = [
            qkp.tile([128, T], BF16, tag=f"yT{j}", name=f"yT{j}")
            for j in range(NPAIR)
        ]
        wp_sb = qkp.tile([128, NPAIR, C], BF16, tag="wp")

        wq_sb = wpool.tile([128, CI, JL], BF16, tag="wq")
        wk_sb = wpool.tile([128, CI, JL], BF16, tag="wk")
        wv_sb = wpool.tile([128, CI, JL], BF16, tag="wv")
        # batched DMAs: DGE issue time (~600ns each) is the startup
        # bottleneck, and a single issue's descriptors fan out across all
        # 16 DMA engines anyway.  wq/xt0 go in two halves so the first
        # projection chain's ci 0-3 matmuls start one transfer earlier.
        xt0 = xpool.tile([128, CI, 512], BF16, tag="xt")
        h = CI // 2
        nc.sync.dma_start(wq_sb[:, 0:h, :], wq_r[:, 0:h, :])
        nc.sync.dma_start(xt0[:, 0:h, :], xT_r[:, 0:h, 0:512])
        nc.sync.dma_start(wq_sb[:, h:CI, :], wq_r[:, h:CI, :])
        nc.sync.dma_start(xt0[:, h:CI, :], xT_r[:, h:CI, 0:512])
        nc.sync.dma_start(wk_sb[:], wk_r[:])
        nc.sync.dma_start(wv_sb[:], wv_r[:])
        nc.sync.dma_start(wp_sb[:], wp_r[:])

        # ---- projection chains for one 512-wide t-chunk -------------------
        def proj_chains(tch, xt):
            ts_ = slice(tch * 512, tch * 512 + 512)
            chains = []

            def qk_chain(w_sb, dst, eng, pr):
                def emit():
                    acc = psf.tile([128, 512], F32, tag="ps", name="pacc")
                    for ci in range(CI):
                        nc.tensor.matmul(
                            acc[:],
                            w_sb[:, ci, pr * 128 : pr * 128 + 128],
                            xt[:, ci, :],
                            start=(ci == 0),
                            stop=(ci == CI - 1),
                        )
                    if eng == "v":
                        nc.vector.tensor_copy(dst[:, pr, ts_], acc[:])
                    else:
                        nc.scalar.copy(dst[:, pr, ts_], acc[:])
                return emit

            def v_chain(tl):
                def emit():
                    ti = tch * 4 + tl
                    acc = psf.tile([128, 512], F32, tag="ps", name="pacc")
                    for ci in range(CI):
                        nc.tensor.matmul(
                            acc[:],
                            xt[:, ci, tl * 128 : tl * 128 + 128],
                            wv_sb[:, ci, :],
                            start=(ci == 0),
                            stop=(ci == CI - 1),
                        )
                    nc.vector.tensor_copy(
                        v_view[:, ti, :, 1 : D + 1],
                        acc[:].rearrange("p (h d) -> p h d", d=D),
                    )
                return emit

            for pr in range(NPAIR):
                chains.append(qk_chain(wq_sb, qT_all, "v", pr))
            for pr in range(NPAIR):
                chains.append(qk_chain(wk_sb, kT_all, "a", pr))
            for tl in range(4):
                chains.append(v_chain(tl))
            return chains

        def proj_block(tch, xt):
            for emit in proj_chains(tch, xt):
                emit()

        # ---- attention machinery -----------------------------------------
        outq = []          # pending out-proj (ti, cc) chains
        n_chain = [0]
        o2_tiles = {}      # ti -> [128, 1024] staging tile (cc=0 allocates)

        def emit_outproj(ti, cc, drain=False):
            tss = slice(ti * 128, ti * 128 + 128)
            cs = slice(cc * 512, cc * 512 + 512)
            acc2 = gp.tile([128, 2, 512], F32, tag="g")
            acc = acc2[:, 0, :]
            for ji in range(NPAIR):
                nc.tensor.matmul(
                    acc,
                    yTs[ji][:, tss],
                    wp_sb[:, ji, cs],
                    start=(ji == 0),
                    stop=(ji == NPAIR - 1),
                )
            # stage both cc halves of a ti into one [128, 1024] tile so the
            # out DMA moves 2KB-contiguous rows (efficient descriptors)
            if ti not in o2_tiles:
                o2_tiles[ti] = outp.tile([128, C], BF16, tag="o", name=f"o2_{ti}")
            o2 = o2_tiles[ti]
            if n_chain[0] % 2 == 0:
                nc.vector.tensor_copy(o2[:, cs], acc)
            else:
                nc.scalar.copy(o2[:, cs], acc)
            n_chain[0] += 1
            if cc == 1:
                # split rows across DGEs so the transfer drains in parallel;
                # the final drain also enlists gpsimd's SWDGE
                r0 = ti * 128
                if drain:
                    nc.sync.dma_start(out[r0 : r0 + 48, :], o2[0:48, :])
                    nc.scalar.dma_start(out[r0 + 48 : r0 + 96, :], o2[48:96, :])
                    nc.gpsimd.dma_start(out[r0 + 96 : r0 + 128, :], o2[96:128, :])
                else:
                    # mid-kernel: keep the issue off ACT (it carries exp +
                    # copies); the sync DGE has headroom here
                    nc.sync.dma_start(out[r0 : r0 + 64, :], o2[0:64, :])
                    nc.sync.dma_start(out[r0 + 64 : r0 + 128, :], o2[64:128, :])
                del o2_tiles[ti]

        # deferred normalize tail: the gpsimd scale-multiply of (qc, pr) is
        # emitted after (qc, pr+1)'s masks so gpsimd's strict FIFO doesn't
        # delay the causal-mask affine_selects behind a 2.3us multiply.
        deferred = []

        def pop_deferred(eng=None):
            sABd, bcd, stgd, pr_, qs_ = deferred.pop(0)
            (eng or nc.gpsimd).tensor_mul(stgd[:], sABd[:], bcd[:])
            nc.sync.dma_start(yTs[pr_][0:64, qs_], stgd[1 : D + 1, 0, :])
            nc.sync.dma_start(yTs[pr_][64:128, qs_], stgd[1 : D + 1, 1, :])

        def attn_block(qc, fill=None):
            fill = fill or []
            qs = slice(qc * 512, qc * 512 + 512)
            for pr in range(NPAIR):
                qlo = qT_all[0:64, pr, :]
                qhi = qT_all[64:128, pr, :]
                klo = kT_all[0:64, pr, :]
                khi = kT_all[64:128, pr, :]
                nkt = 4 * qc + 4
                yA = psf.tile([128, 512], F32, tag="ps")
                yB = psf.tile([128, 512], F32, tag="ps")

                def emit_pv(kt, e, first, last, yA=yA, yB=yB, pr=pr, qc=qc):
                    dt = kt - 4 * qc
                    lo = dt * 128 if dt > 0 else 0
                    nc.tensor.matmul(
                        yA[:, lo:512],
                        v_sb[:, kt, 2 * pr * VW : 2 * pr * VW + 128],
                        e[:, 0, lo:512],
                        start=first,
                        stop=last,
                    )
                    nc.tensor.matmul(
                        yB[:, lo:512],
                        v_sb[:, kt, (2 * pr + 1) * VW : (2 * pr + 1) * VW + 128],
                        e[:, 1, lo:512],
                        start=first,
                        stop=last,
                    )

                def emit_scores(kt, qc=qc, klo=klo, khi=khi, qlo=qlo, qhi=qhi):
                    dt = kt - 4 * qc
                    xlo = dt * 128 if dt > 0 else 0
                    ks = slice(kt * 128, kt * 128 + 128)
                    qw = slice(qc * 512 + xlo, qc * 512 + 512)
                    g = gp.tile([128, 2, 512], F32, tag="g")
                    nc.tensor.matmul(
                        g[:, 0, xlo:512], klo[:, ks], qlo[:, qw],
                        start=True, stop=True,
                    )
                    nc.tensor.matmul(
                        g[:, 1, xlo:512], khi[:, ks], qhi[:, qw],
                        start=True, stop=True,
                    )
                    e = expp.tile([128, 2, 512], BF16, tag="e")
                    if kt % 2 == 0:
                        # true exp on ACT
                        nc.scalar.activation(
                            e[:, :, xlo:512],
                            g[:, :, xlo:512],
                            mybir.ActivationFunctionType.Exp,
                            scale=0.125,
                        )
                    else:
                        # Schraudolph bit-hack exp on DVE
                        nc.vector.tensor_scalar(
                            e[:, :, xlo:512].bitcast(I16),
                            g[:, :, xlo:512],
                            EXP_A,
                            EXP_B,
                            mybir.AluOpType.mult,
                            mybir.AluOpType.add,
                        )
                    if dt >= 0:
                        # zero the causal triangle (k > q) of the diagonal
                        # block, on the otherwise-idle gpsimd engine
                        bs = slice(dt * 128, dt * 128 + 128)
                        for h in (0, 1):
                            nc.gpsimd.affine_select(
                                out=e[:, h, bs],
                                in_=e[:, h, bs],
                                compare_op=mybir.AluOpType.is_ge,
                                fill=0.0,
                                base=0,
                                pattern=[[1, 128]],
                                channel_multiplier=-1,
                            )
                    return e

                # kt order: the 4 diagonal kts FIRST (their serial
                # exp->mask->PV chain overlaps the pr-start bubble), then
                # the full-width kts, so the pr tail drains without gpsimd
                # masks on the critical path.  Steps of 2: both scores
                # pairs back-to-back on the PE queue, then both lagged PV
                # pairs, halving rhs-stream-switch refills (~105ns each).
                # diag kts narrowest-first: the pr-start latency chain
                # (scores -> exp -> mask -> first PV) rides the 128-wide
                # tile instead of the 512-wide one
                kt_order = list(range(4 * qc + 3, 4 * qc - 1, -1)) + list(
                    range(4 * qc)
                )
                pending = []
                n_emitted = [0]

                def drain_one():
                    kt, e = pending.pop(0)
                    emit_pv(kt, e, n_emitted[0] == 0, n_emitted[0] == nkt - 1)
                    n_emitted[0] += 1

                for i0 in range(0, nkt, 2):
                    for kt in kt_order[i0 : i0 + 2]:
                        pending.append((kt, emit_scores(kt)))
                    # fill the pipeline-fill bubble at pr start with
                    # out-proj chains (pr >= 1: the previous attn block's
                    # pr-3 normalize tail may still be in flight at pr 0),
                    # a reserved chain from two blocks ago, or a projection
                    # chain (dependency-free).
                    if i0 == 0:
                        if pr >= 1:
                            for _ in range(2):
                                if outq:
                                    emit_outproj(*outq.pop(0))
                        elif fill:
                            fill.pop(0)()
                    while len(pending) > 3:
                        drain_one()
                    if i0 == 4 and deferred:
                        # the previous pr's deferred gpsimd scale-multiply:
                        # emitted here so this pr's causal masks (all queued
                        # by i0==4, diag-first) precede it in gpsimd's FIFO
                        pop_deferred()
                    if outq and i0 % 4 == 2 and (pr >= 1 or i0 >= 6):
                        emit_outproj(*outq.pop(0))
                while pending:
                    drain_one()

                # normalize: y / rowsum (sums live on psum partition 0).
                rc = rcp.tile([1, 2, 512], F32, tag="rc")
                stg = stp.tile([D + 1, 2, 512], BF16, tag="stg")
                if qc == NQC - 1 and pr == NPAIR - 1:
                    # last normalize of the kernel: the final out-proj drain
                    # waits on it.  Skip the psum-release copies (nothing
                    # needs the slots after this), skip the DRAM-bounce
                    # broadcast (2 DMA hops) -- broadcast the reciprocals
                    # across partitions with a contraction-1 matmul into
                    # psum, and scale on DVE instead of gpsimd.
                    while deferred:
                        pop_deferred()
                    sAB = sab.tile([D + 1, 2, 512], F32, tag="s")
                    nc.scalar.copy(sAB[:, 0, :], yA[0 : D + 1, :])
                    nc.vector.tensor_copy(sAB[:, 1, :], yB[0 : D + 1, :])
                    nc.vector.reciprocal_approx_fast(
                        rc[0:1, 0, :], sAB[0:1, 0, :]
                    )
                    nc.vector.reciprocal_approx_fast(
                        rc[0:1, 1, :], sAB[0:1, 1, :]
                    )
                    rcb = rcp.tile([1, 2, 512], BF16, tag="rcb")
                    nc.scalar.copy(rcb[:], rc[:])
                    bc2 = gp.tile([128, 2, 512], F32, tag="g")
                    for h in (0, 1):
                        nc.tensor.matmul(
                            bc2[0 : D + 1, h, :],
                            ones_bc[0:1, :],
                            rcb[0:1, h, :],
                            start=True,
                            stop=True,
                        )
                    nc.vector.tensor_mul(
                        stg[:], sAB[:], bc2[0 : D + 1, :, :]
                    )
                    nc.sync.dma_start(yTs[pr][0:64, qs], stg[1 : D + 1, 0, :])
                    nc.scalar.dma_start(
                        yTs[pr][64:128, qs], stg[1 : D + 1, 1, :]
                    )
                else:
                    # Copies release the yA/yB psum slots the next pr's
                    # first PVs wait on -- split across ACT and DVE.
                    sAB = sab.tile([D + 1, 2, 512], F32, tag="s")
                    nc.scalar.copy(sAB[:, 0, :], yA[0 : D + 1, :])
                    nc.vector.tensor_copy(sAB[:, 1, :], yB[0 : D + 1, :])
                    nc.vector.reciprocal_approx_fast(
                        rc[0:1, 0, :], sAB[0:1, 0, :]
                    )
                    nc.vector.reciprocal_approx_fast(
                        rc[0:1, 1, :], sAB[0:1, 1, :]
                    )
                    bc = bcp.tile([D + 1, 2, 512], F32, tag="bc")
                    for h in (0, 1):
                        nc.sync.dma_start(
                            rcd[pr, qc, h : h + 1, :], rc[0:1, h, :]
                        )
                        s = rcd[pr, qc, h, :]
                        src = bass.AP(
                            tensor=s.tensor,
                            offset=s.offset,
                            ap=[[0, D + 1]] + list(s.ap),
                        )
                        nc.sync.dma_start(bc[0 : D + 1, h, :], src)
                    deferred.append((sAB, bc, stg, pr, qs))
                    if qc == 0 and len(deferred) > 1:
                        # qc0's prs are too short for the bounce round-trip:
                        # a gpsimd mul here would still be queued when the
                        # next pr's masks arrive, so use DVE
                        pop_deferred(nc.vector)

                # fill pr-boundary bubbles with projection chains for a
                # later t-chunk (PE-dense, no attention dependencies)
                for _ in range(3):
                    if fill:
                        fill.pop(0)()

            while fill:
                fill.pop(0)()
            # block-end flush runs on DVE: a gpsimd mul here would block the
            # next block's causal masks in gpsimd's strict FIFO
            while deferred:
                pop_deferred(nc.vector)
            for ti in range(qc * 4, qc * 4 + 4):
                for cc in range(2):
                    outq.append((ti, cc))

        # ---- interleaved schedule ----------------------------------------
        # tch0, tch1, qc0, tch2, qc1, tch3, qc2, qc3: attention qc only
        # needs projection chunks <= qc; projection blocks are PE-bound
        # while attention is ACT/DVE-heavy, so this keeps every engine fed.
        xts = [xt0, None, None, None]

        def load_x(tch):
            xt = xpool.tile([128, CI, 512], BF16, tag="xt")
            ts_ = slice(tch * 512, tch * 512 + 512)
            nc.sync.dma_start(xt[:], xT_r[:, :, ts_])
            return xt

        xts[1] = load_x(1)
        proj_block(0, xts[0])
        xts[2] = load_x(2)
        proj_block(1, xts[1])
        xts[3] = load_x(3)
        attn_block(0, fill=proj_chains(2, xts[2]))
        p3 = proj_chains(3, xts[3])
        # hold back two tch3 Q/K chains to fill qc2's pr-0 bubble (safe:
        # qc3 reads them much later; V chains are needed at qc3's start)
        attn_block(1, fill=p3[:6] + p3[8:])
        attn_block(2, fill=p3[6:8])
        attn_block(3)

        while outq:
            emit_outproj(*outq.pop(0), drain=True)

    nc.finalize()
    return nc


def _get_nc():
    global _CACHED_NC
    if _CACHED_NC is None:
        _CACHED_NC = build_nc()
    return _CACHED_NC


def kernel(x, Wq, Wk, Wv, Wp):
    import ml_dtypes
    from concourse.bass_utils import run_bass_kernel_spmd

    BF = ml_dtypes.bfloat16
    x = np.asarray(x, dtype=np.float32)
    Wq = np.asarray(Wq, dtype=np.float32)
    Wk = np.asarray(Wk, dtype=np.float32)
    Wv = np.asarray(Wv, dtype=np.float32)
    Wp = np.asarray(Wp, dtype=np.float32)

    nc = _get_nc()

    xT = [np.ascontiguousarray(x[b].T).astype(BF) for b in range(B)]
    wqT, wkT, wvT, wpT = [], [], [], []
    for hh in range(2):
        js = slice(JL * hh, JL * hh + JL)
        wqT.append(np.ascontiguousarray(Wq[js, :].T).astype(BF))
        wkT.append(np.ascontiguousarray(Wk[js, :].T).astype(BF))
        wvT.append(np.ascontiguousarray(Wv[js, :].T).astype(BF))
        wpT.append(np.ascontiguousarray(Wp[:, js].T).astype(BF))

    in_maps = []
    for c in range(NCORES):
        b, hh = c // 2, c % 2
        in_maps.append(
            {
                "xT": xT[b],
                "wqT": wqT[hh],
                "wkT": wkT[hh],
                "wvT": wvT[hh],
                "wpT": wpT[hh],
            }
        )

    res = run_bass_kernel_spmd(nc, in_maps, core_ids=list(range(NCORES)))

    out = np.empty((B, T, C), dtype=np.float32)
    for b in range(B):
        out[b] = res.results[2 * b]["out"].astype(np.float32) + res.results[
            2 * b + 1
        ]["out"].astype(np.float32)
    return out


# revision 91
# speedup vs baseline: 1.0267x; 1.0267x over previous
"""Causal self-attention Bass kernel for 8 TRN2 NeuronCores.

Problem: B=4, T=2048, C=1024, H=16 heads, head_dim=64, fp32.
    q = x @ Wq.T ; k = x @ Wk.T ; v = x @ Wv.T          (per head)
    att = softmax(mask(q k^T / 8))
    y = att @ v ; out = y @ Wp.T

Sharding (8 cores): 4-way data parallel over batch x 2-way tensor
parallel over heads. Core c handles batch c//2 and heads 8*(c%2)..+8.
Wq/Wk/Wv column-parallel, Wp row-parallel; the partial outputs of the
two head-halves of each batch are summed on the host (the "all-reduce"
of row-parallel Wp).

Pipeline design (v7):
  - Projections stream xT by 512-wide t-chunks (tch).  The causal
    structure means attention q-chunk qc only needs projections from
    chunks <= qc, so attention blocks INTERLEAVE with projection
    blocks: tch0, tch1, qc0, tch2, qc1, tch3, qc2, qc3.  The
    projection blocks are PE-bound while attention is ACT/DVE-heavy,
    so interleaving keeps every engine fed.
  - exp alternates between ACT (even kt, true exp) and DVE (odd kt,
    Schraudolph bf16 bit-hack: bits = round(s*23.083 + 16248.5) as
    int16 reinterpreted bf16; ~1.8% rms on half the weights, ~5e-3
    added rel err after softmax averaging).
  - PV lags exp by 3 kts; scores/PV emitted in 2-kt batches to halve
    rhs-stream-switch pipeline refills.
  - Diagonal kts run FIRST within each (qc, pr) so the serial
    exp->mask(gpsimd)->PV chain overlaps the pr-start bubble and the
    pr tail drains mask-free.
  - Output projection chains interleave into later attention blocks.
  - Softmax normalize: sums (ones-column of the PV stationary) ->
    ACT/DVE copies release the psum accumulators early -> DVE
    reciprocal -> DRAM-bounce broadcast -> gpsimd scale (deferred one
    pr so gpsimd's FIFO doesn't delay causal masks).
  - Everything bf16: FWL keeps LDWEIGHTS off the critical path, input
    DMA traffic halves.  fp32 accumulation throughout.
"""

from contextlib import ExitStack

import numpy as np

import concourse.bass as bass
import concourse.tile as tile
from concourse import bacc, mybir

F32 = mybir.dt.float32
BF16 = mybir.dt.bfloat16
I16 = mybir.dt.int16

B, T, C, H, D = 4, 2048, 1024, 16, 64
NCORES = 8
JL = 512            # local j dims per core (8 heads * 64)
NPAIR = 4           # local head pairs
CI = C // 128       # 8 c-tiles
NT = T // 128       # 16 t/k tiles
NQC = T // 512      # 4 q chunks
VW = D + 1

# Schraudolph bf16 bit-hack exp: bits = s*0.125*log2(e)*128 + (127*128 - 7.5)
EXP_A = 0.125 * 1.4426950408889634 * 128.0
EXP_B = 127.0 * 128.0 - 7.5

_CACHED_NC = None


def build_nc():
    nc = bacc.Bacc(None)

    xT = nc.dram_tensor("xT", [C, T], BF16, kind="ExternalInput")
    wqT = nc.dram_tensor("wqT", [C, JL], BF16, kind="ExternalInput")
    wkT = nc.dram_tensor("wkT", [C, JL], BF16, kind="ExternalInput")
    wvT = nc.dram_tensor("wvT", [C, JL], BF16, kind="ExternalInput")
    wpT = nc.dram_tensor("wpT", [JL, C], BF16, kind="ExternalInput")
    out = nc.dram_tensor("out", [T, C], BF16, kind="ExternalOutput")
    # bounce buffer for broadcasting softmax reciprocals across partitions
    rcd = nc.dram_tensor("rcd", [NPAIR, NQC, 2, 512], F32)

    xT_r = xT.rearrange("(ci p) t -> p ci t", p=128)
    wq_r = wqT.rearrange("(ci p) j -> p ci j", p=128)
    wk_r = wkT.rearrange("(ci p) j -> p ci j", p=128)
    wv_r = wvT.rearrange("(ci p) j -> p ci j", p=128)
    wp_r = wpT.rearrange("(ji p) c -> p ji c", p=128)

    with tile.TileContext(nc) as tc, ExitStack() as ctx:
        pm = ctx.enter_context(tc.tile_pool(name="pm", bufs=1))
        qkp = ctx.enter_context(tc.tile_pool(name="qkp", bufs=1))
        expp = ctx.enter_context(tc.tile_pool(name="expp", bufs=5))
        bcp = ctx.enter_context(tc.tile_pool(name="bcp", bufs=2))
        rcp = ctx.enter_context(tc.tile_pool(name="rcp", bufs=2))
        sab = ctx.enter_context(tc.tile_pool(name="sab", bufs=2))
        stp = ctx.enter_context(tc.tile_pool(name="stp", bufs=2))
        outp = ctx.enter_context(tc.tile_pool(name="outp", bufs=3))
        wpool = ctx.enter_context(tc.tile_pool(name="ph1w", bufs=1))
        xpool = ctx.enter_context(tc.tile_pool(name="ph1x", bufs=2))
        # PSUM: scores (f32, 2 banks each) x3 + shared f32 ring x2 = 8 banks
        gp = ctx.enter_context(tc.tile_pool(name="gp", bufs=3, space="PSUM"))
        psf = ctx.enter_context(tc.tile_pool(name="psf", bufs=2, space="PSUM"))

        # v with a ones column prepended per head (so the softmax sums land
        # on psum partition 0), plus 64 pad columns so every per-head lhsT
        # can be read as [128, 128] -- NumWeights==128 enables FWL.
        v_sb = pm.tile([128, NT, 8 * VW + 64], BF16)
        v_view = v_sb[:, :, 0 : 8 * VW].rearrange("p n (h w) -> p n h w", w=VW)
        ones_col = pm.tile([128, NT, 8, 1], F32)
        nc.vector.memset(ones_col[:], 1.0)
        nc.vector.tensor_copy(v_view[:, :, :, 0:1], ones_col[:])
        nc.vector.memset(v_sb[:, :, 8 * VW : 8 * VW + 64], 0.0)
        # [1, 65] ones: stationary operand of the contraction-1 broadcast
        # matmul used by the last normalize
        ones_bc = pm.tile([1, D + 1], BF16)
        nc.vector.memset(ones_bc[:], 1.0)

        qT_all = qkp.tile([128, NPAIR, T], BF16, tag="qT_all")
        kT_all = qkp.tile([128, NPAIR, T], BF16, tag="kT_all")
        # one yT tile per head-pair: precise dependency tracking, so an
        # out-proj chain's ji<3 matmuls never wait on a fresher pr's yT
        yTs = [
            qkp.tile([128, T], BF16, tag=f"yT{j}", name=f"yT{j}")
            for j in range(NPAIR)
        ]
        wp_sb = qkp.tile([128, NPAIR, C], BF16, tag="wp")

        wq_sb = wpool.tile([128, CI, JL], BF16, tag="wq")
        wk_sb = wpool.tile([128, CI, JL], BF16, tag="wk")
        wv_sb = wpool.tile([128, CI, JL], BF16, tag="wv")
        # batched DMAs: DGE issue time (~600ns each) is the startup
        # bottleneck, and a single issue's descriptors fan out across all
        # 16 DMA engines anyway.  wq/xt0 go in two halves so the first
        # projection chain's ci 0-3 matmuls start one transfer earlier.
        xt0 = xpool.tile([128, CI, 512], BF16, tag="xt")
        h = CI // 2
        nc.sync.dma_start(wq_sb[:, 0:h, :], wq_r[:, 0:h, :])
        nc.sync.dma_start(xt0[:, 0:h, :], xT_r[:, 0:h, 0:512])
        nc.sync.dma_start(wq_sb[:, h:CI, :], wq_r[:, h:CI, :])
        nc.sync.dma_start(xt0[:, h:CI, :], xT_r[:, h:CI, 0:512])
        nc.sync.dma_start(wk_sb[:], wk_r[:])
        nc.sync.dma_start(wv_sb[:], wv_r[:])
        nc.sync.dma_start(wp_sb[:], wp_r[:])

        # ---- projection chains for one 512-wide t-chunk -------------------
        def proj_chains(tch, xt):
            ts_ = slice(tch * 512, tch * 512 + 512)
            chains = []

            def qk_chain(w_sb, dst, eng, pr):
                def emit():
                    acc = psf.tile([128, 512], F32, tag="ps", name="pacc")
                    for ci in range(CI):
                        nc.tensor.matmul(
                            acc[:],
                            w_sb[:, ci, pr * 128 : pr * 128 + 128],
                            xt[:, ci, :],
                            start=(ci == 0),
                            stop=(ci == CI - 1),
                        )
                    if eng == "v":
                        nc.vector.tensor_copy(dst[:, pr, ts_], acc[:])
                    else:
                        nc.scalar.copy(dst[:, pr, ts_], acc[:])
                return emit

            def v_chain(tl):
                def emit():
                    ti = tch * 4 + tl
                    acc = psf.tile([128, 512], F32, tag="ps", name="pacc")
                    for ci in range(CI):
                        nc.tensor.matmul(
                            acc[:],
                            xt[:, ci, tl * 128 : tl * 128 + 128],
                            wv_sb[:, ci, :],
                            start=(ci == 0),
                            stop=(ci == CI - 1),
                        )
                    nc.vector.tensor_copy(
                        v_view[:, ti, :, 1 : D + 1],
                        acc[:].rearrange("p (h d) -> p h d", d=D),
                    )
                return emit

            for pr in range(NPAIR):
                chains.append(qk_chain(wq_sb, qT_all, "v", pr))
            for pr in range(NPAIR):
                chains.append(qk_chain(wk_sb, kT_all, "a", pr))
            for tl in range(4):
                chains.append(v_chain(tl))
            return chains

        def proj_block(tch, xt):
            for emit in proj_chains(tch, xt):
                emit()

        # ---- attention machinery -----------------------------------------
        outq = []          # pending out-proj (ti, cc) chains
        n_chain = [0]
        o2_tiles = {}      # ti -> [128, 1024] staging tile (cc=0 allocates)

        def emit_outproj(ti, cc, drain=False):
            tss = slice(ti * 128, ti * 128 + 128)
            cs = slice(cc * 512, cc * 512 + 512)
            acc2 = gp.tile([128, 2, 512], F32, tag="g")
            acc = acc2[:, 0, :]
            for ji in range(NPAIR):
                nc.tensor.matmul(
                    acc,
                    yTs[ji][:, tss],
                    wp_sb[:, ji, cs],
                    start=(ji == 0),
                    stop=(ji == NPAIR - 1),
                )
            # stage both cc halves of a ti into one [128, 1024] tile so the
            # out DMA moves 2KB-contiguous rows (efficient descriptors)
            if ti not in o2_tiles:
                o2_tiles[ti] = outp.tile([128, C], BF16, tag="o", name=f"o2_{ti}")
            o2 = o2_tiles[ti]
            if n_chain[0] % 2 == 0:
                nc.vector.tensor_copy(o2[:, cs], acc)
            else:
                nc.scalar.copy(o2[:, cs], acc)
            n_chain[0] += 1
            if cc == 1:
                # split rows across DGEs so the transfer drains in parallel;
                # the final drain also enlists gpsimd's SWDGE
                r0 = ti * 128
                if drain:
                    nc.sync.dma_start(out[r0 : r0 + 48, :], o2[0:48, :])
                    nc.scalar.dma_start(out[r0 + 48 : r0 + 96, :], o2[48:96, :])
                    nc.gpsimd.dma_start(out[r0 + 96 : r0 + 128, :], o2[96:128, :])
                else:
                    # mid-kernel: keep the issue off ACT (it carries exp +
                    # copies); the sync DGE has headroom here
                    nc.sync.dma_start(out[r0 : r0 + 64, :], o2[0:64, :])
                    nc.sync.dma_start(out[r0 + 64 : r0 + 128, :], o2[64:128, :])
                del o2_tiles[ti]

        # deferred normalize tail: the gpsimd scale-multiply of (qc, pr) is
        # emitted after (qc, pr+1)'s masks so gpsimd's strict FIFO doesn't
        # delay the causal-mask affine_selects behind a 2.3us multiply.
        deferred = []

        def pop_deferred(eng=None):
            sABd, bcd, stgd, pr_, qs_ = deferred.pop(0)
            (eng or nc.gpsimd).tensor_mul(stgd[:], sABd[:], bcd[:])
            nc.sync.dma_start(yTs[pr_][0:64, qs_], stgd[1 : D + 1, 0, :])
            nc.sync.dma_start(yTs[pr_][64:128, qs_], stgd[1 : D + 1, 1, :])

        def attn_block(qc, fill=None):
            fill = fill or []
            qs = slice(qc * 512, qc * 512 + 512)
            for pr in range(NPAIR):
                qlo = qT_all[0:64, pr, :]
                qhi = qT_all[64:128, pr, :]
                klo = kT_all[0:64, pr, :]
                khi = kT_all[64:128, pr, :]
                nkt = 4 * qc + 4
                yA = psf.tile([128, 512], F32, tag="ps")
                yB = psf.tile([128, 512], F32, tag="ps")

                def emit_pv(kt, e, first, last, yA=yA, yB=yB, pr=pr, qc=qc):
                    dt = kt - 4 * qc
                    lo = dt * 128 if dt > 0 else 0
                    nc.tensor.matmul(
                        yA[:, lo:512],
                        v_sb[:, kt, 2 * pr * VW : 2 * pr * VW + 128],
                        e[:, 0, lo:512],
                        start=first,
                        stop=last,
                    )
                    nc.tensor.matmul(
                        yB[:, lo:512],
                        v_sb[:, kt, (2 * pr + 1) * VW : (2 * pr + 1) * VW + 128],
                        e[:, 1, lo:512],
                        start=first,
                        stop=last,
                    )

                def emit_scores(kt, qc=qc, klo=klo, khi=khi, qlo=qlo, qhi=qhi):
                    dt = kt - 4 * qc
                    xlo = dt * 128 if dt > 0 else 0
                    ks = slice(kt * 128, kt * 128 + 128)
                    qw = slice(qc * 512 + xlo, qc * 512 + 512)
                    g = gp.tile([128, 2, 512], F32, tag="g")
                    nc.tensor.matmul(
                        g[:, 0, xlo:512], klo[:, ks], qlo[:, qw],
                        start=True, stop=True,
                    )
                    nc.tensor.matmul(
                        g[:, 1, xlo:512], khi[:, ks], qhi[:, qw],
                        start=True, stop=True,
                    )
                    e = expp.tile([128, 2, 512], BF16, tag="e")
                    if kt % 2 == 0:
                        # true exp on ACT
                        nc.scalar.activation(
                            e[:, :, xlo:512],
                            g[:, :, xlo:512],
                            mybir.ActivationFunctionType.Exp,
                            scale=0.125,
                        )
                    else:
                        # Schraudolph bit-hack exp on DVE
                        nc.vector.tensor_scalar(
                            e[:, :, xlo:512].bitcast(I16),
                            g[:, :, xlo:512],
                            EXP_A,
                            EXP_B,
                            mybir.AluOpType.mult,
                            mybir.AluOpType.add,
                        )
                    if dt >= 0:
                        # zero the causal triangle (k > q) of the diagonal
                        # block, on the otherwise-idle gpsimd engine
                        bs = slice(dt * 128, dt * 128 + 128)
                        for h in (0, 1):
                            nc.gpsimd.affine_select(
                                out=e[:, h, bs],
                                in_=e[:, h, bs],
                                compare_op=mybir.AluOpType.is_ge,
                                fill=0.0,
                                base=0,
                                pattern=[[1, 128]],
                                channel_multiplier=-1,
                            )
                    return e

                # kt order: the 4 diagonal kts FIRST (their serial
                # exp->mask->PV chain overlaps the pr-start bubble), then
                # the full-width kts, so the pr tail drains without gpsimd
                # masks on the critical path.  Steps of 2: both scores
                # pairs back-to-back on the PE queue, then both lagged PV
                # pairs, halving rhs-stream-switch refills (~105ns each).
                kt_order = list(range(4 * qc, 4 * qc + 4)) + list(range(4 * qc))
                pending = []
                n_emitted = [0]

                def drain_one():
                    kt, e = pending.pop(0)
                    emit_pv(kt, e, n_emitted[0] == 0, n_emitted[0] == nkt - 1)
                    n_emitted[0] += 1

                for i0 in range(0, nkt, 2):
                    for kt in kt_order[i0 : i0 + 2]:
                        pending.append((kt, emit_scores(kt)))
                    # fill the pipeline-fill bubble at pr start with
                    # out-proj chains (pr >= 1: the previous attn block's
                    # pr-3 normalize tail may still be in flight at pr 0),
                    # a reserved chain from two blocks ago, or a projection
                    # chain (dependency-free).
                    if i0 == 0:
                        if pr >= 1:
                            for _ in range(2):
                                if outq:
                                    emit_outproj(*outq.pop(0))
                        elif fill:
                            fill.pop(0)()
                    while len(pending) > 3:
                        drain_one()
                    if i0 == 4 and deferred:
                        # the previous pr's deferred gpsimd scale-multiply:
                        # emitted here so this pr's causal masks (all queued
                        # by i0==4, diag-first) precede it in gpsimd's FIFO
                        pop_deferred()
                    if outq and i0 % 4 == 2 and (pr >= 1 or i0 >= 6):
                        emit_outproj(*outq.pop(0))
                while pending:
                    drain_one()

                # normalize: y / rowsum (sums live on psum partition 0).
                rc = rcp.tile([1, 2, 512], F32, tag="rc")
                stg = stp.tile([D + 1, 2, 512], BF16, tag="stg")
                if qc == NQC - 1 and pr == NPAIR - 1:
                    # last normalize of the kernel: the final out-proj drain
                    # waits on it.  Skip the psum-release copies (nothing
                    # needs the slots after this), skip the DRAM-bounce
                    # broadcast (2 DMA hops) -- broadcast the reciprocals
                    # across partitions with a contraction-1 matmul into
                    # psum, and scale on DVE instead of gpsimd.
                    while deferred:
                        pop_deferred()
                    sAB = sab.tile([D + 1, 2, 512], F32, tag="s")
                    nc.scalar.copy(sAB[:, 0, :], yA[0 : D + 1, :])
                    nc.vector.tensor_copy(sAB[:, 1, :], yB[0 : D + 1, :])
                    nc.vector.reciprocal_approx_fast(
                        rc[0:1, 0, :], sAB[0:1, 0, :]
                    )
                    nc.vector.reciprocal_approx_fast(
                        rc[0:1, 1, :], sAB[0:1, 1, :]
                    )
                    rcb = rcp.tile([1, 2, 512], BF16, tag="rcb")
                    nc.scalar.copy(rcb[:], rc[:])
                    bc2 = gp.tile([128, 2, 512], F32, tag="g")
                    for h in (0, 1):
                        nc.tensor.matmul(
                            bc2[0 : D + 1, h, :],
                            ones_bc[0:1, :],
                            rcb[0:1, h, :],
                            start=True,
                            stop=True,
                        )
                    nc.vector.tensor_mul(
                        stg[:], sAB[:], bc2[0 : D + 1, :, :]
                    )
                    nc.sync.dma_start(yTs[pr][0:64, qs], stg[1 : D + 1, 0, :])
                    nc.scalar.dma_start(
                        yTs[pr][64:128, qs], stg[1 : D + 1, 1, :]
                    )
                else:
                    # Copies release the yA/yB psum slots the next pr's
                    # first PVs wait on -- split across ACT and DVE.
                    sAB = sab.tile([D + 1, 2, 512], F32, tag="s")
                    nc.scalar.copy(sAB[:, 0, :], yA[0 : D + 1, :])
                    nc.vector.tensor_copy(sAB[:, 1, :], yB[0 : D + 1, :])
                    nc.vector.reciprocal_approx_fast(
                        rc[0:1, 0, :], sAB[0:1, 0, :]
                    )
                    nc.vector.reciprocal_approx_fast(
                        rc[0:1, 1, :], sAB[0:1, 1, :]
                    )
                    bc = bcp.tile([D + 1, 2, 512], F32, tag="bc")
                    for h in (0, 1):
                        nc.sync.dma_start(
                            rcd[pr, qc, h : h + 1, :], rc[0:1, h, :]
                        )
                        s = rcd[pr, qc, h, :]
                        src = bass.AP(
                            tensor=s.tensor,
                            offset=s.offset,
                            ap=[[0, D + 1]] + list(s.ap),
                        )
                        nc.sync.dma_start(bc[0 : D + 1, h, :], src)
                    deferred.append((sAB, bc, stg, pr, qs))
                    if qc == 0 and len(deferred) > 1:
                        # qc0's prs are too short for the bounce round-trip:
                        # a gpsimd mul here would still be queued when the
                        # next pr's masks arrive, so use DVE
                        pop_deferred(nc.vector)

                # fill pr-boundary bubbles with projection chains for a
                # later t-chunk (PE-dense, no attention dependencies)
                for _ in range(3):
                    if fill:
                        fill.pop(0)()

            while fill:
                fill.pop(0)()
            # block-end flush runs on DVE: a gpsimd mul here would block the
            # next block's causal masks in gpsimd's strict FIFO
            while deferred:
                pop_deferred(nc.vector)
            for ti in range(qc * 4, qc * 4 + 4):
                for cc in range(2):
                    outq.append((ti, cc))

        # ---- interleaved schedule ----------------------------------------
        # tch0, tch1, qc0, tch2, qc1, tch3, qc2, qc3: attention qc only
        # needs projection chunks <= qc; projection blocks are PE-bound
        # while attention is ACT/DVE-heavy, so this keeps every engine fed.
        xts = [xt0, None, None, None]

        def load_x(tch):
            xt = xpool.tile([128, CI, 512], BF16, tag="xt")
            ts_ = slice(tch * 512, tch * 512 + 512)
            nc.sync.dma_start(xt[:], xT_r[:, :, ts_])
            return xt

        xts[1] = load_x(1)
        proj_block(0, xts[0])
        xts[2] = load_x(2)
        proj_block(1, xts[1])
        xts[3] = load_x(3)
        attn_block(0, fill=proj_chains(2, xts[2]))
        p3 = proj_chains(3, xts[3])
        # hold back two tch3 Q/K chains to fill qc2's pr-0 bubble (safe:
        # qc3 reads them much later; V chains are needed at qc3's start)
        attn_block(1, fill=p3[:6] + p3[8:])
        attn_block(2, fill=p3[6:8])
        attn_block(3)

        while outq:
            emit_outproj(*outq.pop(0), drain=True)

    nc.finalize()
    return nc


def _get_nc():
    global _CACHED_NC
    if _CACHED_NC is None:
        _CACHED_NC = build_nc()
    return _CACHED_NC


def kernel(x, Wq, Wk, Wv, Wp):
    import ml_dtypes
    from concourse.bass_utils import run_bass_kernel_spmd

    BF = ml_dtypes.bfloat16
    x = np.asarray(x, dtype=np.float32)
    Wq = np.asarray(Wq, dtype=np.float32)
    Wk = np.asarray(Wk, dtype=np.float32)
    Wv = np.asarray(Wv, dtype=np.float32)
    Wp = np.asarray(Wp, dtype=np.float32)

    nc = _get_nc()

    xT = [np.ascontiguousarray(x[b].T).astype(BF) for b in range(B)]
    wqT, wkT, wvT, wpT = [], [], [], []
    for hh in range(2):
        js = slice(JL * hh, JL * hh + JL)
        wqT.append(np.ascontiguousarray(Wq[js, :].T).astype(BF))
        wkT.append(np.ascontiguousarray(Wk[js, :].T).astype(BF))
        wvT.append(np.ascontiguousarray(Wv[js, :].T).astype(BF))
        wpT.append(np.ascontiguousarray(Wp[:, js].T).astype(BF))

    in_maps = []
    for c in range(NCORES):
        b, hh = c // 2, c % 2
        in_maps.append(
            {
                "xT": xT[b],
                "wqT": wqT[hh],
                "wkT": wkT[hh],
                "wvT": wvT[hh],
                "wpT": wpT[hh],
            }
        )

    res = run_bass_kernel_spmd(nc, in_maps, core_ids=list(range(NCORES)))

    out = np.empty((B, T, C), dtype=np.float32)
    for b in range(B):
        out[b] = res.results[2 * b]["out"].astype(np.float32) + res.results[
            2 * b + 1
        ]["out"].astype(np.float32)
    return out
